# revision 8
# baseline (speedup 1.0000x reference)
"""Bass/Trainium2 kernel for fused bilinear attention + softmax.

reference computation:
    pa = a @ Wa + ba                      (B, La, D)
    pb = b @ Wb + bb                      (B, Lb, D)
    scores = einsum('bid,bjd->bij', pa * w, pb) + wbias
    out = softmax(scores.reshape(B, La*Lb)).reshape(B, La, Lb)

Device strategy (8 NeuronCores, data-parallel over batch, 8 batches/core):
    Weight-only host folding:  M = (Wa*w) @ Wb.T,  u = (Wa*w)@bb,  v = (Wb*w)@ba
      scores[b,i,j] = a_i M b_j^T + (a_i.u) + (b_j.v) + const
    const (+wbias) dropped: softmax over the flattened grid is shift-invariant.
    bu[b,j] = v . b_j is a rank-1 term computed on host (like u/v folding).

    Everything device-side carries a power-of-2 scale sM on M (so the fp8
    chunks use the e4m3 range); exp() unscales via its scale operand.

    Per pair of batches (rhs free dim 512):
      TT   = (sM*M) @ bT + sM*u   mixed-precision contraction:
             first N8 feature chunks as fp8e4m3 DoubleRow pair-matmuls
             (2 chunks per instruction, 2x PE throughput), the rest bf16.
             DVE eviction to bf16 (split per batch half) adds sM*u.
      S    = aT^T @ TT + 1(x)(sM*bu)  bf16 matmuls (N=256) + K=1 inject
      softmax: per-half ACT exp(S/sM) with accum_out rowsum (first half
               overlaps the second half's score matmuls) -> one f32
               ones-matmul on PE broadcasts both rowsums -> DVE add+recip ->
               DVE scale -> per-half DMA out on alternating queues
    Group-0 latency: input DMAs are balanced across the Sync and Scalar
    HWDGE queues (each sustains ~200 GB/s independently) and group 0 runs
    its fp8 DoubleRow matmuls for m=0..5 before any bf16 work, so the PE
    starts as soon as the fp8 descriptors land.
    PE warm-up matmuls run during the initial DMAs (HAM clock-gate release).
"""

import numpy as np
import ml_dtypes

import concourse.bass as bass
import concourse.bacc as bacc
import concourse.mybir as mybir
import concourse.tile as tile
from concourse.bass_utils import run_bass_kernel_spmd

BF16 = ml_dtypes.bfloat16
FP8 = ml_dtypes.float8_e4m3      # TRN e4m3: max normal 240

N_CORES = 8
B, L, K = 64, 256, 1024          # batch, seq len (La=Lb), feature dim
BPC = B // N_CORES               # batches per core
G = BPC // 2                     # batch-pair groups per core
KC = K // 128                    # feature chunks of 128
N8 = 6                           # fp8 feature chunks (rest bf16): 6/8 split
C8 = N8 // 2                     # DoubleRow pair-instructions per m-chunk
NB16 = KC - N8                   # bf16 feature chunks
F32 = mybir.dt.float32
DBF = mybir.dt.bfloat16
F8 = mybir.dt.float8e4
Act = mybir.ActivationFunctionType
PM = mybir.MatmulPerfMode


def _build_program(sm_inv):
    # Bacc (not raw Bass): its compile() legalizes multi-wait instructions
    # (TRN2 allows at most one sync wait per instruction).
    nc = bacc.Bacc("TRN2", debug=False, target_bir_lowering=False)

    at = nc.dram_tensor("at", [G, 128, KC, 2 * L], DBF, kind="ExternalInput")
    bt8 = nc.dram_tensor("bt8", [G, 128, C8, 2, 2 * L], F8, kind="ExternalInput")
    bt16 = nc.dram_tensor("bt16", [G, 128, NB16, 2 * L], DBF, kind="ExternalInput")
    mt8 = nc.dram_tensor("mt8", [4, 128, 2, C8, 2, 128], F8, kind="ExternalInput")
    mt16 = nc.dram_tensor("mt16", [2, 128, KC // 2, NB16, 128], DBF, kind="ExternalInput")
    u = nc.dram_tensor("u", [128, KC], F32, kind="ExternalInput")
    bu = nc.dram_tensor("bu", [1, BPC * L], DBF, kind="ExternalInput")
    probs = nc.dram_tensor("probs", [BPC, L, L], F32, kind="ExternalOutput")

    with tile.TileContext(nc) as tc:
        with (
            tc.tile_pool(name="consts", bufs=1) as consts,
            tc.tile_pool(name="inp", bufs=2) as in_pool,
            tc.tile_pool(name="tt", bufs=10) as tt_pool,
            tc.tile_pool(name="sm", bufs=4) as sm_pool,
            tc.tile_pool(name="small", bufs=4) as small,
            tc.tile_pool(name="ps_tt", bufs=6, space="PSUM") as ps_tt,
            tc.tile_pool(name="ps_sc", bufs=2, space="PSUM") as ps_sc,
        ):
            mt8_t = [consts.tile([128, 2, C8, 2, 128], F8, name=f"mt8_{j}") for j in range(4)]
            mt16_t = [consts.tile([128, KC // 2, NB16, 128], DBF, name=f"mt16_{j}") for j in range(2)]
            u_sb = consts.tile([128, KC], F32)
            bu_sb = consts.tile([1, BPC * L], DBF)
            ones_row_bf = consts.tile([1, 128], DBF)
            nc.vector.memset(ones_row_bf, 1.0)
            ones_sq_f32 = consts.tile([128, 128], F32)
            nc.vector.memset(ones_sq_f32, 1.0)

            # PE warm-up: dummy matmuls while the first DMAs land, so the HAM
            # clock gate is already released when real matmuls start.
            warm_sb = consts.tile([128, 128], DBF)
            nc.gpsimd.memset(warm_sb, 0.0)
            warm_ps = ps_sc.tile([128, 2 * L], F32, tag="sc")
            for i in range(3):
                nc.tensor.matmul(
                    warm_ps[:, 0:128], warm_sb, warm_sb,
                    start=(i == 0), stop=(i == 2),
                )

            def evict(tt_ps, m):
                """PSUM -> bf16 SBUF (+ sM*u[m]), split per batch half so the
                q=0 scores can start before the q=1 half is evicted."""
                tt_sb = tt_pool.tile([128, 2 * L], DBF, tag="tt")
                for q in range(2):
                    nc.vector.tensor_scalar_add(
                        tt_sb[:, q * L : (q + 1) * L],
                        tt_ps[:, q * L : (q + 1) * L],
                        u_sb[:, m : m + 1],
                    )
                return tt_sb

            def mm8(tt_ps, m, bt8_sb, c):
                nc.tensor.matmul(
                    tt_ps, mt8_t[m // 2][:, m % 2, c], bt8_sb[:, c],
                    start=(c == 0), stop=False,
                    perf_mode=PM.DoubleRow, skip_group_check=True,
                )

            def mm16(tt_ps, m, bt16_sb, l):
                nc.tensor.matmul(
                    tt_ps, mt16_t[m // 4][:, m % 4, l], bt16_sb[:, l],
                    start=False, stop=(l == NB16 - 1),
                    skip_group_check=True,
                )

            for g in range(G):
                bt8_sb = in_pool.tile([128, C8, 2, 2 * L], F8, tag="bt8")
                bt16_sb = in_pool.tile([128, NB16, 2 * L], DBF, tag="bt16")
                at_t = [
                    in_pool.tile(
                        [128, KC // 2, 2 * L], DBF, tag=f"at{j}", name=f"at_{j}"
                    )
                    for j in range(2)
                ]
                if g == 0:
                    # Balanced two-queue fill. Sync: fp8 b + bf16 b + bf16 M.
                    # Scalar: fp8 M in 4 small descs (they gate the DR
                    # sweep and sit behind the ACT table load), u/bu, a.
                    nc.sync.dma_start(out=bt8_sb, in_=bt8[g])
                    for j in range(4):
                        nc.scalar.dma_start(out=mt8_t[j], in_=mt8[j])
                    nc.sync.dma_start(out=bt16_sb, in_=bt16[g])
                    for j in range(2):
                        nc.sync.dma_start(out=mt16_t[j], in_=mt16[j])
                    nc.scalar.dma_start(out=u_sb, in_=u[:, :])
                    nc.scalar.dma_start(out=bu_sb, in_=bu[:, :])
                    for j in range(2):
                        nc.scalar.dma_start(
                            out=at_t[j],
                            in_=at[g][:, j * (KC // 2) : (j + 1) * (KC // 2)],
                        )
                else:
                    nc.sync.dma_start(out=bt8_sb, in_=bt8[g])
                    nc.sync.dma_start(out=bt16_sb, in_=bt16[g])
                    nc.sync.dma_start(
                        out=at_t[0], in_=at[g][:, 0 : KC // 2]
                    )
                    nc.scalar.dma_start(
                        out=at_t[1], in_=at[g][:, KC // 2 : KC]
                    )

                # Phase 1: all 8 TT chunks (kept in SBUF; tt_pool holds them).
                tt_chunks = []
                if g == 0:
                    # fp8 sweep m=0..5 first (gated only on the small fp8
                    # DMAs), then the bf16 passes; m=6,7 run interleaved on
                    # the first two recycled ps_tt banks.
                    ps6 = []
                    for m in range(6):
                        tt_ps = ps_tt.tile([128, 2 * L], F32, tag="tt_ps")
                        ps6.append(tt_ps)
                        for c in range(C8):
                            mm8(tt_ps, m, bt8_sb, c)
                    for m in range(6):
                        for l in range(NB16):
                            mm16(ps6[m], m, bt16_sb, l)
                        tt_chunks.append(evict(ps6[m], m))
                    for m in range(6, KC):
                        tt_ps = ps_tt.tile([128, 2 * L], F32, tag="tt_ps")
                        for c in range(C8):
                            mm8(tt_ps, m, bt8_sb, c)
                        for l in range(NB16):
                            mm16(tt_ps, m, bt16_sb, l)
                        tt_chunks.append(evict(tt_ps, m))
                else:
                    for m in range(KC):
                        tt_ps = ps_tt.tile([128, 2 * L], F32, tag="tt_ps")
                        for c in range(C8):
                            mm8(tt_ps, m, bt8_sb, c)
                        for l in range(NB16):
                            mm16(tt_ps, m, bt16_sb, l)
                        tt_chunks.append(evict(tt_ps, m))

                # Phase 2: scores per batch in ONE psum bank (sequential h
                # accumulation groups), softmax split by half so the first
                # half's exp overlaps the second half's matmuls.
                for q in range(2):
                    bq = 2 * g + q
                    sc_ps = ps_sc.tile([128, 2 * L], F32, tag="sc")
                    exp_sb = sm_pool.tile([128, 2, L], F32, tag="exp")
                    colsum = small.tile([128, 2], F32, tag="cs")
                    for h in range(2):
                        for m in range(KC):
                            nc.tensor.matmul(
                                sc_ps[:, h * L : (h + 1) * L],
                                at_t[m // 4][
                                    :, m % 4,
                                    q * L + h * 128 : q * L + h * 128 + 128,
                                ],
                                tt_chunks[m][:, q * L : (q + 1) * L],
                                start=(m == 0), stop=False,
                            )
                        # inject bu (K=1 accumulate): S[i, j] += 1 * sM*bu[j]
                        nc.tensor.matmul(
                            sc_ps[:, h * L : (h + 1) * L],
                            ones_row_bf, bu_sb[:, bq * L : (bq + 1) * L],
                            start=False, stop=True,
                        )
                        nc.scalar.activation(
                            exp_sb[:, h], sc_ps[:, h * L : (h + 1) * L],
                            Act.Exp, scale=float(sm_inv),
                            accum_out=colsum[:, h : h + 1],
                        )

                    # total over partitions: f32 ones-matmuls sum both halves'
                    # rowsums and broadcast to every partition (the h=0 one
                    # issues early, so only the h=1 accumulate waits on exp).
                    # Fresh ps_tt tile — its ring slot is long recycled.
                    tot_ps = ps_tt.tile([128, 2 * L], F32, tag="tt_ps")
                    for h in range(2):
                        nc.tensor.matmul(
                            tot_ps[:, 0:1], ones_sq_f32, colsum[:, h : h + 1],
                            start=(h == 0), stop=(h == 1),
                            skip_group_check=True,
                        )
                    rcp_col = small.tile([128, 1], F32, tag="rcpc")
                    nc.vector.reciprocal(rcp_col, tot_ps[:, 0:1])
                    probs_sb = sm_pool.tile([128, 2, L], F32, tag="probs")
                    for h in range(2):
                        # split by half so the first DMA overlaps the second mul
                        nc.vector.tensor_scalar_mul(
                            probs_sb[:, h], exp_sb[:, h], rcp_col
                        )
                        nc.sync.dma_start(
                            out=probs[bq][h * 128 : (h + 1) * 128, :],
                            in_=probs_sb[:, h],
                        )
    return nc


def _prep_host(a, b, Wa, ba, Wb, bb, w, wbias):
    """Weight folding (f64) + per-core shards: mixed fp8/bf16 feature-major."""
    Wa64 = Wa.astype(np.float64)
    Wb64 = Wb.astype(np.float64)
    w64 = w.astype(np.float64)
    M = (Wa64 * w64[None, :]) @ Wb64.T                  # (K, K)
    u64 = (Wa64 * w64[None, :]) @ bb.astype(np.float64)
    v64 = (Wb64 * w64[None, :]) @ ba.astype(np.float64)

    sM = 2.0 ** np.floor(np.log2(239.0 / np.abs(M).max()))
    Ms = M * sM                                          # scaled fold

    # mt8[j, p, m', c, i, km] = sM*M[(4j+m')*128+km, (2c+i)*128+p]
    # mt16[j, p, m', l, km]   = sM*M[(4j+m')*128+km, (N8+l)*128+p]
    Mb = Ms.reshape(KC, 128, KC, 128)                    # [m, km, lc, p]
    mt8_np = np.ascontiguousarray(
        Mb[:, :, :N8, :]
        .reshape(4, 2, 128, C8, 2, 128)
        .transpose(0, 5, 1, 3, 4, 2)
    ).astype(FP8)
    mt16_np = np.ascontiguousarray(
        Mb[:, :, N8:, :]
        .reshape(2, KC // 2, 128, NB16, 128)
        .transpose(0, 4, 1, 3, 2)
    ).astype(BF16)

    u_np = np.ascontiguousarray(
        (u64 * sM).astype(np.float32).reshape(KC, 128).T
    )                                                    # [p, c]

    # bu[b, j] = v . b[b, j, :], host rank-1 fold (scaled)
    bu_all = (b.astype(np.float64) @ v64) * sM           # (B, L)

    in_maps = []
    for cidx in range(N_CORES):
        sl = slice(cidx * BPC, (cidx + 1) * BPC)
        a_c, b_c = a[sl], b[sl]
        # feature-major, batch pairs side by side: x_fm[g, k, q*L+j]
        def fm(x):
            xt = x.transpose(0, 2, 1)                    # (BPC, K, L)
            return xt.reshape(G, 2, K, L).transpose(0, 2, 1, 3).reshape(G, K, 2 * L)
        a_fm = fm(a_c)
        b_fm = fm(b_c)
        at_np = np.ascontiguousarray(
            a_fm.reshape(G, KC, 128, 2 * L).transpose(0, 2, 1, 3)
        ).astype(BF16)
        b8 = b_fm[:, : N8 * 128, :].reshape(G, C8, 2, 128, 2 * L)
        bt8_np = np.ascontiguousarray(b8.transpose(0, 3, 1, 2, 4)).astype(FP8)
        b16 = b_fm[:, N8 * 128 :, :].reshape(G, NB16, 128, 2 * L)
        bt16_np = np.ascontiguousarray(b16.transpose(0, 2, 1, 3)).astype(BF16)
        bu_np = np.ascontiguousarray(
            bu_all[sl].reshape(1, BPC * L)
        ).astype(BF16)
        in_maps.append(
            {
                "at": at_np,
                "bt8": bt8_np,
                "bt16": bt16_np,
                "mt8": mt8_np,
                "mt16": mt16_np,
                "u": u_np,
                "bu": bu_np,
            }
        )
    return in_maps, 1.0 / sM


def _run(inputs, trace=False):
    in_maps, sm_inv = _prep_host(**inputs)
    nc = _build_program(sm_inv)
    nc.compile()
    res = run_bass_kernel_spmd(
        nc, in_maps, core_ids=list(range(N_CORES)), trace=trace
    )
    out = np.concatenate([res.results[c]["probs"] for c in range(N_CORES)], axis=0)
    return out.astype(np.float32), res


def kernel(**inputs) -> np.ndarray:
    out, _ = _run(inputs, trace=False)
    return out


# revision 9
# speedup vs baseline: 1.1889x; 1.1889x over previous
"""Bass/Trainium2 kernel for fused bilinear attention + softmax.

reference computation:
    pa = a @ Wa + ba                      (B, La, D)
    pb = b @ Wb + bb                      (B, Lb, D)
    scores = einsum('bid,bjd->bij', pa * w, pb) + wbias
    out = softmax(scores.reshape(B, La*Lb)).reshape(B, La, Lb)

Device strategy (8 NeuronCores, data-parallel over batch, 8 batches/core):
    Weight-only host folding:  M = (Wa*w) @ Wb.T,  u = (Wa*w)@bb,  v = (Wb*w)@ba
      scores[b,i,j] = a_i M b_j^T + (a_i.u) + (b_j.v) + const
    const (+wbias) dropped: softmax over the flattened grid is shift-invariant.
    bu[b,j] = v . b_j is a rank-1 term computed on host (like u/v folding).

    Everything device-side carries a power-of-2 scale sM on M (so the fp8
    chunks use the e4m3 range); exp() unscales via its scale operand.

    Per pair of batches (rhs free dim 512):
      TT   = (sM*M) @ bT + sM*u   mixed-precision contraction:
             first N8 feature chunks as fp8e4m3 DoubleRow pair-matmuls
             (2 chunks per instruction, 2x PE throughput), the rest bf16.
             DVE eviction to bf16 (split per batch half) adds sM*u.
      S    = aT^T @ TT + 1(x)(sM*bu)  bf16 matmuls (N=256) + K=1 inject
      softmax: per-half ACT exp(S/sM) with accum_out rowsum (first half
               overlaps the second half's score matmuls) -> one f32
               ones-matmul on PE broadcasts both rowsums -> DVE add+recip ->
               DVE scale -> per-half DMA out on alternating queues
    Group-0 latency: input DMAs are balanced across the Sync and Scalar
    HWDGE queues (each sustains ~200 GB/s independently) and group 0 runs
    its fp8 DoubleRow matmuls for m=0..5 before any bf16 work, so the PE
    starts as soon as the fp8 descriptors land.
    PE warm-up matmuls run during the initial DMAs (HAM clock-gate release).
"""

import numpy as np
import ml_dtypes

import concourse.bass as bass
import concourse.bacc as bacc
import concourse.mybir as mybir
import concourse.tile as tile
from concourse.bass_utils import run_bass_kernel_spmd

BF16 = ml_dtypes.bfloat16
FP8 = ml_dtypes.float8_e4m3      # TRN e4m3: max normal 240

N_CORES = 8
B, L, K = 64, 256, 1024          # batch, seq len (La=Lb), feature dim
BPC = B // N_CORES               # batches per core
G = BPC // 2                     # batch-pair groups per core
KC = K // 128                    # feature chunks of 128
N8 = 6                           # fp8 feature chunks (rest bf16): 6/8 split
C8 = N8 // 2                     # DoubleRow pair-instructions per m-chunk
NB16 = KC - N8                   # bf16 feature chunks
F32 = mybir.dt.float32
DBF = mybir.dt.bfloat16
F8 = mybir.dt.float8e4
Act = mybir.ActivationFunctionType
PM = mybir.MatmulPerfMode


def _build_program(sm_inv):
    # Bacc (not raw Bass): its compile() legalizes multi-wait instructions
    # (TRN2 allows at most one sync wait per instruction).
    nc = bacc.Bacc("TRN2", debug=False, target_bir_lowering=False)

    at = nc.dram_tensor("at", [G, 128, KC, 2 * L], DBF, kind="ExternalInput")
    bt8 = nc.dram_tensor("bt8", [G, 128, C8, 2, 2 * L], F8, kind="ExternalInput")
    bt16 = nc.dram_tensor("bt16", [G, 128, NB16, 2 * L], DBF, kind="ExternalInput")
    mt8 = nc.dram_tensor("mt8", [4, 128, 2, C8, 2, 128], F8, kind="ExternalInput")
    mt16 = nc.dram_tensor("mt16", [2, 128, KC // 2, NB16, 128], DBF, kind="ExternalInput")
    u = nc.dram_tensor("u", [128, KC], F32, kind="ExternalInput")
    bu = nc.dram_tensor("bu", [1, BPC * L], DBF, kind="ExternalInput")
    probs = nc.dram_tensor("probs", [BPC, L, L], F32, kind="ExternalOutput")

    with tile.TileContext(nc) as tc:
        with (
            tc.tile_pool(name="consts", bufs=1) as consts,
            tc.tile_pool(name="inp", bufs=2) as in_pool,
            tc.tile_pool(name="tt", bufs=10) as tt_pool,
            tc.tile_pool(name="sm", bufs=4) as sm_pool,
            tc.tile_pool(name="small", bufs=4) as small,
            tc.tile_pool(name="ps_tt", bufs=6, space="PSUM") as ps_tt,
            tc.tile_pool(name="ps_sc", bufs=2, space="PSUM") as ps_sc,
        ):
            mt8_t = [consts.tile([128, 2, C8, 2, 128], F8, name=f"mt8_{j}") for j in range(4)]
            mt16_t = [consts.tile([128, KC // 2, NB16, 128], DBF, name=f"mt16_{j}") for j in range(2)]
            u_sb = consts.tile([128, KC], F32)
            bu_sb = consts.tile([1, BPC * L], DBF)
            ones_row_bf = consts.tile([1, 128], DBF)
            nc.vector.memset(ones_row_bf, 1.0)
            ones_sq_f32 = consts.tile([128, 128], F32)
            nc.vector.memset(ones_sq_f32, 1.0)

            # PE warm-up: dummy matmuls while the first DMAs land, so the HAM
            # clock gate is already released when real matmuls start.
            warm_sb = consts.tile([128, 128], DBF)
            nc.vector.memset(warm_sb, 0.0)
            warm_ps = ps_sc.tile([128, 2 * L], F32, tag="sc")
            for i in range(3):
                nc.tensor.matmul(
                    warm_ps[:, 0:128], warm_sb, warm_sb,
                    start=(i == 0), stop=(i == 2),
                )

            def evict(tt_ps, m):
                """PSUM -> bf16 SBUF (+ sM*u[m]), split per batch half so the
                q=0 scores can start before the q=1 half is evicted."""
                tt_sb = tt_pool.tile([128, 2 * L], DBF, tag="tt")
                for q in range(2):
                    nc.vector.tensor_scalar_add(
                        tt_sb[:, q * L : (q + 1) * L],
                        tt_ps[:, q * L : (q + 1) * L],
                        u_sb[:, m : m + 1],
                    )
                return tt_sb

            def mm8(tt_ps, m, bt8_sb, c):
                nc.tensor.matmul(
                    tt_ps, mt8_t[m // 2][:, m % 2, c], bt8_sb[:, c],
                    start=(c == 0), stop=False,
                    perf_mode=PM.DoubleRow, skip_group_check=True,
                )

            def mm16(tt_ps, m, bt16_sb, l):
                nc.tensor.matmul(
                    tt_ps, mt16_t[m // 4][:, m % 4, l], bt16_sb[:, l],
                    start=False, stop=(l == NB16 - 1),
                    skip_group_check=True,
                )

            for g in range(G):
                bt8_sb = in_pool.tile([128, C8, 2, 2 * L], F8, tag="bt8")
                bt16_sb = in_pool.tile([128, NB16, 2 * L], DBF, tag="bt16")
                at_t = [
                    in_pool.tile(
                        [128, KC // 2, 2 * L], DBF, tag=f"at{j}", name=f"at_{j}"
                    )
                    for j in range(2)
                ]
                if g == 0:
                    # Balanced two-queue fill. Sync: fp8 b + bf16 b + bf16 M.
                    # Scalar: fp8 M in 4 small descs (they gate the DR
                    # sweep and sit behind the ACT table load), u/bu, a.
                    nc.sync.dma_start(out=bt8_sb, in_=bt8[g])
                    for j in range(4):
                        nc.scalar.dma_start(out=mt8_t[j], in_=mt8[j])
                    nc.sync.dma_start(out=bt16_sb, in_=bt16[g])
                    for j in range(2):
                        nc.sync.dma_start(out=mt16_t[j], in_=mt16[j])
                    nc.scalar.dma_start(out=u_sb, in_=u[:, :])
                    nc.scalar.dma_start(out=bu_sb, in_=bu[:, :])
                    for j in range(2):
                        nc.scalar.dma_start(
                            out=at_t[j],
                            in_=at[g][:, j * (KC // 2) : (j + 1) * (KC // 2)],
                        )
                else:
                    nc.sync.dma_start(out=bt8_sb, in_=bt8[g])
                    nc.sync.dma_start(out=bt16_sb, in_=bt16[g])
                    nc.sync.dma_start(
                        out=at_t[0], in_=at[g][:, 0 : KC // 2]
                    )
                    nc.scalar.dma_start(
                        out=at_t[1], in_=at[g][:, KC // 2 : KC]
                    )

                # Phase 1: all 8 TT chunks (kept in SBUF; tt_pool holds them).
                tt_chunks = []
                if g == 0:
                    # fp8 sweep m=0..5 first (gated only on the small fp8
                    # DMAs), then the bf16 passes; m=6,7 run interleaved on
                    # the first two recycled ps_tt banks.
                    ps6 = []
                    for m in range(6):
                        tt_ps = ps_tt.tile([128, 2 * L], F32, tag="tt_ps")
                        ps6.append(tt_ps)
                        for c in range(C8):
                            mm8(tt_ps, m, bt8_sb, c)
                    for m in range(6):
                        for l in range(NB16):
                            mm16(ps6[m], m, bt16_sb, l)
                        tt_chunks.append(evict(ps6[m], m))
                    for m in range(6, KC):
                        tt_ps = ps_tt.tile([128, 2 * L], F32, tag="tt_ps")
                        for c in range(C8):
                            mm8(tt_ps, m, bt8_sb, c)
                        for l in range(NB16):
                            mm16(tt_ps, m, bt16_sb, l)
                        tt_chunks.append(evict(tt_ps, m))
                else:
                    for m in range(KC):
                        tt_ps = ps_tt.tile([128, 2 * L], F32, tag="tt_ps")
                        for c in range(C8):
                            mm8(tt_ps, m, bt8_sb, c)
                        for l in range(NB16):
                            mm16(tt_ps, m, bt16_sb, l)
                        tt_chunks.append(evict(tt_ps, m))

                # Phase 2: scores per batch in ONE psum bank (sequential h
                # accumulation groups), softmax split by half so the first
                # half's exp overlaps the second half's matmuls.
                for q in range(2):
                    bq = 2 * g + q
                    sc_ps = ps_sc.tile([128, 2 * L], F32, tag="sc")
                    exp_sb = sm_pool.tile([128, 2, L], F32, tag="exp")
                    colsum = small.tile([128, 2], F32, tag="cs")
                    for h in range(2):
                        for m in range(KC):
                            nc.tensor.matmul(
                                sc_ps[:, h * L : (h + 1) * L],
                                at_t[m // 4][
                                    :, m % 4,
                                    q * L + h * 128 : q * L + h * 128 + 128,
                                ],
                                tt_chunks[m][:, q * L : (q + 1) * L],
                                start=(m == 0), stop=False,
                            )
                        # inject bu (K=1 accumulate): S[i, j] += 1 * sM*bu[j]
                        nc.tensor.matmul(
                            sc_ps[:, h * L : (h + 1) * L],
                            ones_row_bf, bu_sb[:, bq * L : (bq + 1) * L],
                            start=False, stop=True,
                        )
                        nc.scalar.activation(
                            exp_sb[:, h], sc_ps[:, h * L : (h + 1) * L],
                            Act.Exp, scale=float(sm_inv),
                            accum_out=colsum[:, h : h + 1],
                        )

                    # total over partitions: f32 ones-matmuls sum both halves'
                    # rowsums and broadcast to every partition (the h=0 one
                    # issues early, so only the h=1 accumulate waits on exp).
                    # Fresh ps_tt tile — its ring slot is long recycled.
                    tot_ps = ps_tt.tile([128, 2 * L], F32, tag="tt_ps")
                    for h in range(2):
                        nc.tensor.matmul(
                            tot_ps[:, 0:1], ones_sq_f32, colsum[:, h : h + 1],
                            start=(h == 0), stop=(h == 1),
                            skip_group_check=True,
                        )
                    rcp_col = small.tile([128, 1], F32, tag="rcpc")
                    nc.vector.reciprocal(rcp_col, tot_ps[:, 0:1])
                    probs_sb = sm_pool.tile([128, 2, L], F32, tag="probs")
                    for h in range(2):
                        # split by half so the first DMA overlaps the second mul
                        nc.vector.tensor_scalar_mul(
                            probs_sb[:, h], exp_sb[:, h], rcp_col
                        )
                        nc.sync.dma_start(
                            out=probs[bq][h * 128 : (h + 1) * 128, :],
                            in_=probs_sb[:, h],
                        )
    return nc


def _prep_host(a, b, Wa, ba, Wb, bb, w, wbias):
    """Weight folding (f64) + per-core shards: mixed fp8/bf16 feature-major."""
    Wa64 = Wa.astype(np.float64)
    Wb64 = Wb.astype(np.float64)
    w64 = w.astype(np.float64)
    M = (Wa64 * w64[None, :]) @ Wb64.T                  # (K, K)
    u64 = (Wa64 * w64[None, :]) @ bb.astype(np.float64)
    v64 = (Wb64 * w64[None, :]) @ ba.astype(np.float64)

    sM = 2.0 ** np.floor(np.log2(239.0 / np.abs(M).max()))
    Ms = M * sM                                          # scaled fold

    # mt8[j, p, m', c, i, km] = sM*M[(4j+m')*128+km, (2c+i)*128+p]
    # mt16[j, p, m', l, km]   = sM*M[(4j+m')*128+km, (N8+l)*128+p]
    Mb = Ms.reshape(KC, 128, KC, 128)                    # [m, km, lc, p]
    mt8_np = np.ascontiguousarray(
        Mb[:, :, :N8, :]
        .reshape(4, 2, 128, C8, 2, 128)
        .transpose(0, 5, 1, 3, 4, 2)
    ).astype(FP8)
    mt16_np = np.ascontiguousarray(
        Mb[:, :, N8:, :]
        .reshape(2, KC // 2, 128, NB16, 128)
        .transpose(0, 4, 1, 3, 2)
    ).astype(BF16)

    u_np = np.ascontiguousarray(
        (u64 * sM).astype(np.float32).reshape(KC, 128).T
    )                                                    # [p, c]

    # bu[b, j] = v . b[b, j, :], host rank-1 fold (scaled)
    bu_all = (b.astype(np.float64) @ v64) * sM           # (B, L)

    in_maps = []
    for cidx in range(N_CORES):
        sl = slice(cidx * BPC, (cidx + 1) * BPC)
        a_c, b_c = a[sl], b[sl]
        # feature-major, batch pairs side by side: x_fm[g, k, q*L+j]
        def fm(x):
            xt = x.transpose(0, 2, 1)                    # (BPC, K, L)
            return xt.reshape(G, 2, K, L).transpose(0, 2, 1, 3).reshape(G, K, 2 * L)
        a_fm = fm(a_c)
        b_fm = fm(b_c)
        at_np = np.ascontiguousarray(
            a_fm.reshape(G, KC, 128, 2 * L).transpose(0, 2, 1, 3)
        ).astype(BF16)
        b8 = b_fm[:, : N8 * 128, :].reshape(G, C8, 2, 128, 2 * L)
        bt8_np = np.ascontiguousarray(b8.transpose(0, 3, 1, 2, 4)).astype(FP8)
        b16 = b_fm[:, N8 * 128 :, :].reshape(G, NB16, 128, 2 * L)
        bt16_np = np.ascontiguousarray(b16.transpose(0, 2, 1, 3)).astype(BF16)
        bu_np = np.ascontiguousarray(
            bu_all[sl].reshape(1, BPC * L)
        ).astype(BF16)
        in_maps.append(
            {
                "at": at_np,
                "bt8": bt8_np,
                "bt16": bt16_np,
                "mt8": mt8_np,
                "mt16": mt16_np,
                "u": u_np,
                "bu": bu_np,
            }
        )
    return in_maps, 1.0 / sM


def _run(inputs, trace=False):
    in_maps, sm_inv = _prep_host(**inputs)
    nc = _build_program(sm_inv)
    nc.compile()
    res = run_bass_kernel_spmd(
        nc, in_maps, core_ids=list(range(N_CORES)), trace=trace
    )
    out = np.concatenate([res.results[c]["probs"] for c in range(N_CORES)], axis=0)
    return out.astype(np.float32), res


def kernel(**inputs) -> np.ndarray:
    out, _ = _run(inputs, trace=False)
    return out
